# revision 7
# baseline (speedup 1.0000x reference)
"""ChebConvNet (K=1) Trainium2 kernel: 3x silu(x@W+b) -> logits -> log_softmax.

Sharding: data-parallel over nodes across 8 NeuronCores. x is padded from
200000 to 200704 rows (8 * 25088); each core processes its shard in a
transposed [feat, node] layout so the 128-wide feature dim sits on SBUF
partitions. The scalar (ACT) engine is the bottleneck (~1 col/cycle @
1.2 GHz for every silu/exp element), so the schedule keeps it saturated:

- whole-shard xT staged in SBUF: graduated chunks over the sync HWDGE
  queue for the early columns, one scalar-queue chunk and one gpsimd
  (SWDGE) chunk for the late columns, all issued at t=0;
- a dummy silu at program start pulls the silu ACT-table load off the
  critical path (the table patch below also pins the memzero's Copy to
  the silu set so only one load is emitted);
- three silu layers as streaming phases (1536-col PSUM macro tiles = 3
  banks x2 + 1 bank x2 for the z matmuls); the z (W3) matmuls run one
  macro tile behind the silu stream so the PE never stalls ACT;
- log_softmax uses hardware-measured fast paths only: exp reads z
  g-major via a (free) strided AP and writes bf16 c-major chunks; the
  per-group sum runs on the idle PE as 40 accumulating identity
  matmuls per chunk (PSUM does the reduction); ln reads PSUM; the
  final subtract is an all-bf16 vector op; exp/ln interleave with one
  table switch (get_activation_tables patched so walrus serves both
  from natural_log_exp_and_others);
- z and the output are bf16 (rel err ~4e-3 vs budget 2e-2), halving
  output DMA, which streams over the sync + scalar queues.

The device writes output partition-major; the host unscrambles and
upcasts to f32. edge_index is unused (ChebConv with K=1 ignores it).
"""

import numpy as np

import concourse.bacc as bacc
import concourse.mybir as mybir
import concourse.tile as tile
from concourse.tile import add_dep_helper
from concourse.bass_utils import run_bass_kernel_spmd

P = 128          # feature dim == SBUF partitions
C = 40           # classes
N_FULL = 200000
N_CORES = 8
NS = 25088       # nodes per core (padded: 8 * 25088 = 200704)
MT = 1536        # macro tile (nodes), 3 PSUM banks
MACROS = [MT] * 16 + [512]            # 16*1536 + 512 = 25088
NG = NS // P                          # 196 node groups per core
# xT staging chunks: (cols, queue) — 0 = sync, 1 = scalar, 2 = gpsimd.
XCHUNKS = [(1536, 0), (3072, 0), (4608, 0), (4608, 0), (6144, 1), (5120, 2)]
assert sum(c for c, _ in XCHUNKS) == NS
GB = 28                               # log_softmax chunk (node groups)
NB = NG // GB                         # 7 chunks
assert NB * GB == NG

F32 = mybir.dt.float32
BF16 = mybir.dt.bfloat16
AF = mybir.ActivationFunctionType

_CACHE = {}


def _patch_act_tables():
    """Pin exp+ln to the joint natural_log_exp set (one switch for the
    softmax tail) and Copy to the silu set (so the table-preload dummy
    costs a single load)."""
    if getattr(bacc, "_act_tables_patched", False):
        return
    orig = bacc.get_activation_tables

    def patched(arch):
        tabs = {k: set(v) for k, v in orig(arch).items()}
        for name, fns in tabs.items():
            if name != "natural_log_exp_and_others":
                fns.discard(AF.Exp)
                fns.discard(AF.Ln)
            if name != "silu_and_others":
                fns.discard(AF.Copy)
        return tabs

    bacc.get_activation_tables = patched
    bacc._act_tables_patched = True


def _build():
    if "nc" in _CACHE:
        return _CACHE["nc"]
    _patch_act_tables()
    nc = bacc.Bacc(None, target_bir_lowering=False)
    xT = nc.declare_dram_parameter("xT", [P, NS], BF16, isOutput=False)
    # consts per partition: identity (256B) + 3 W (256B bf16) + W3 (80B)
    # + 3 bias f32 + b3rep (12*C f32)
    CB = 2 * P + 3 * 2 * P + 2 * C + 3 * 4 + 4 * 12 * C  # 3036 B/partition
    cd = nc.declare_dram_parameter("consts", [P, CB], mybir.dt.uint8, isOutput=False)
    # partition-major scratch layout; host unscrambles to [NS, C]
    out = nc.declare_dram_parameter("out", [P, NG * C], BF16, isOutput=True)

    with tile.TileContext(nc) as tc:
        with (
            tc.tile_pool(name="const", bufs=1) as cpool,
            tc.tile_pool(name="big", bufs=1) as bigp,
            tc.tile_pool(name="h2s", bufs=2) as h2sp,
            tc.tile_pool(name="ob", bufs=3) as obp,
            tc.tile_pool(name="ph", bufs=2, space="PSUM") as ph,
            tc.tile_pool(name="pz", bufs=2, space="PSUM") as pz,
        ):
            # ---- t=0: silu table preload via dummy activation (scalar-only)
            dz = cpool.tile([P, 8], F32, tag="dz")
            nc.scalar.memzero(dz[:, 0:4])
            nc.scalar.activation(dz[:, 4:8], dz[:, 0:4], AF.Silu)

            # ---- consts ----
            craw = cpool.tile([P, CB], mybir.dt.uint8, tag="craw")
            nc.sync.dma_start(craw[:], cd[:])
            off = 0
            ident = craw[:, off : off + 2 * P].bitcast(BF16)
            off += 2 * P
            Wt = []
            for i in range(3):
                Wt.append(craw[:, off : off + 2 * P].bitcast(BF16))
                off += 2 * P
            W3t = craw[:, off : off + 2 * C].bitcast(BF16)
            off += 2 * C
            bt = []
            for i in range(3):
                bt.append(craw[:, off : off + 4].bitcast(F32))
                off += 4
            b3rep = craw[:, off : off + 4 * 12 * C].bitcast(F32)

            # ---- whole-shard staging; h1 reuses xT's slot, exp reuses h0's
            xall = bigp.tile([P, NS], BF16, tag="bigX", name="xall")
            h0 = bigp.tile([P, NS], BF16, tag="bigA", name="h0all")
            zall = bigp.tile([P, NG * C], BF16, tag="zall")
            lsall = bigp.tile([P, NG], BF16, tag="lsall")

            # ---- xT chunk DMAs over sync/scalar/gpsimd queues ----
            qeng = [nc.sync, nc.scalar, nc.gpsimd]
            c0 = 0
            for cw, q in XCHUNKS:
                qeng[q].dma_start(xall[:, c0 : c0 + cw], xT[:, c0 : c0 + cw])
                c0 += cw

            # ---- Phase A0: h0 = silu(x @ W0 + b0) ----
            n0 = 0
            for mt in MACROS:
                hp = ph.tile([P, MT], F32, tag="hpsum", name="hp0")
                for j in range(0, mt, 512):
                    nc.tensor.matmul(
                        hp[:, j : j + 512], Wt[0], xall[:, n0 + j : n0 + j + 512],
                        start=True, stop=True,
                    )
                nc.scalar.activation(
                    h0[:, n0 : n0 + mt], hp[:, :mt], AF.Silu,
                    bias=bt[0], scale=1.0,
                )
                n0 += mt

            # ---- Phase A1: h1 = silu(h0 @ W1 + b1) (h1 aliases xall) ----
            h1 = bigp.tile([P, NS], BF16, tag="bigX", name="h1all")
            n0 = 0
            for mt in MACROS:
                hp = ph.tile([P, MT], F32, tag="hpsum", name="hp1")
                for j in range(0, mt, 512):
                    nc.tensor.matmul(
                        hp[:, j : j + 512], Wt[1], h0[:, n0 + j : n0 + j + 512],
                        start=True, stop=True,
                    )
                nc.scalar.activation(
                    h1[:, n0 : n0 + mt], hp[:, :mt], AF.Silu,
                    bias=bt[1], scale=1.0,
                )
                n0 += mt

            # ---- Phase A2: h2 = silu(h1 @ W2 + b2); z = h2 @ W3 + b3 ----
            # z-matmuls run one macro tile behind silu so PE never stalls ACT
            pend = None  # (h2_tile, n0, mt) awaiting z-matmuls
            last_silu = None

            def emit_z(h2t, zn0, zmt):
                g0, gn = zn0 // P, zmt // P
                zp = pz.tile([P, 12 * C], F32, tag="zpsum")
                for g in range(gn):
                    nc.tensor.matmul(
                        zp[:, g * C : (g + 1) * C],
                        h2t[:, g * P : (g + 1) * P],
                        W3t,
                        start=True, stop=True,
                    )
                nc.vector.tensor_add(
                    zall[:, g0 * C : (g0 + gn) * C],
                    zp[:, : gn * C],
                    b3rep[:, : gn * C],
                )

            n0 = 0
            for mt in MACROS:
                hp = ph.tile([P, MT], F32, tag="hpsum", name="hp2")
                for j in range(0, mt, 512):
                    nc.tensor.matmul(
                        hp[:, j : j + 512], Wt[2], h1[:, n0 + j : n0 + j + 512],
                        start=True, stop=True,
                    )
                h2 = h2sp.tile([P, MT], BF16, tag="h2")
                last_silu = nc.scalar.activation(
                    h2[:, :mt], hp[:, :mt], AF.Silu, bias=bt[2], scale=1.0
                )
                if pend is not None:
                    emit_z(*pend)
                pend = (h2, n0, mt)
                n0 += mt
            emit_z(*pend)

            # ---- Phase B: log_softmax ----
            # exp: strided read of g-major z -> bf16 c-major chunk (full
            # ACT speed); group-sums on the PE via accumulating identity
            # matmuls into PSUM; ln from PSUM -> bf16; bf16 subtract.
            eallT = bigp.tile([P, NG * C], BF16, tag="bigA", name="eallT")

            exps = []
            for k in range(NB):
                g0 = k * GB
                e = nc.scalar.activation(
                    eallT[:, g0 * C : (g0 + GB) * C].rearrange(
                        "p (c g) -> p c g", c=C
                    ),
                    zall[:, g0 * C : (g0 + GB) * C].rearrange(
                        "p (g c) -> p c g", g=GB
                    ),
                    AF.Exp,
                )
                exps.append((e, g0))
            add_dep_helper(exps[0][0].ins, last_silu.ins, sync=True,
                           reason="exp after all silus (ACT table switch)")

            out_q = [nc.sync, nc.scalar]
            for k, (e, g0) in enumerate(exps):
                sp = ph.tile([P, GB], F32, tag="hpsum", name="spsum")
                base = g0 * C
                for c in range(C):
                    nc.tensor.matmul(
                        sp[:],
                        ident,
                        eallT[:, base + c * GB : base + (c + 1) * GB],
                        start=(c == 0), stop=(c == C - 1),
                    )
                nc.scalar.activation(lsall[:, g0 : g0 + GB], sp[:], AF.Ln)
                o = obp.tile([P, GB * C], BF16, tag="o")
                nc.vector.tensor_tensor(
                    o[:].rearrange("p (g c) -> p g c", g=GB),
                    zall[:, base : base + GB * C].rearrange(
                        "p (g c) -> p g c", g=GB
                    ),
                    lsall[:, g0 : g0 + GB].broadcast_to([P, GB, C]),
                    op=mybir.AluOpType.subtract,
                )
                out_q[k % 2].dma_start(out[:, base : base + GB * C], o[:])
    nc.compile()
    _CACHE["nc"] = nc
    return nc


def _in_maps(x, W0, b0, W1, b1, W2, b2, W3, b3):
    import ml_dtypes

    x = np.asarray(x, dtype=np.float32)
    xpad = np.zeros((N_CORES * NS, P), dtype=ml_dtypes.bfloat16)
    xpad[:N_FULL] = x
    parts = [
        np.eye(P, dtype=ml_dtypes.bfloat16).view(np.uint8),
        np.asarray(W0, np.float32).astype(ml_dtypes.bfloat16).view(np.uint8),
        np.asarray(W1, np.float32).astype(ml_dtypes.bfloat16).view(np.uint8),
        np.asarray(W2, np.float32).astype(ml_dtypes.bfloat16).view(np.uint8),
        np.asarray(W3, np.float32).astype(ml_dtypes.bfloat16).view(np.uint8),
        np.asarray(b0, np.float32).reshape(P, 1).view(np.uint8),
        np.asarray(b1, np.float32).reshape(P, 1).view(np.uint8),
        np.asarray(b2, np.float32).reshape(P, 1).view(np.uint8),
        np.ascontiguousarray(
            np.broadcast_to(np.tile(np.asarray(b3, np.float32), 12), (P, 12 * C))
        ).view(np.uint8),
    ]
    common = {"consts": np.ascontiguousarray(np.concatenate(parts, axis=1))}
    maps = []
    for c in range(N_CORES):
        shard = xpad[c * NS : (c + 1) * NS]
        maps.append({**common, "xT": np.ascontiguousarray(shard.T)})
    return maps


def _unscramble(res):
    # device out: bf16 [128, 196*40] with node = g*128 + p -> f32 [25088, 40]
    outs = []
    for c in range(N_CORES):
        o = res.results[c]["out"].reshape(P, NG, C).astype(np.float32)
        outs.append(np.ascontiguousarray(o.transpose(1, 0, 2)).reshape(NS, C))
    return np.concatenate(outs, axis=0)[:N_FULL]


def kernel(**inputs):
    nc = _build()
    maps = _in_maps(
        inputs["x"],
        inputs["W0"], inputs["b0"],
        inputs["W1"], inputs["b1"],
        inputs["W2"], inputs["b2"],
        inputs["W3"], inputs["b3"],
    )
    res = run_bass_kernel_spmd(nc, maps, list(range(N_CORES)))
    return _unscramble(res)
